# revision 1
# baseline (speedup 1.0000x reference)
"""GAT layer (nn_GAT) on 8 Trainium2 NeuronCores — Bass/Tile SPMD kernel.

Math (per head h):
    Wh   = x @ W[h]                         [N, HID]
    s_i  = Wh_i . a1[h],  d_j = Wh_j . a2[h]
    e_ij = leakyrelu(s_i + d_j, 0.2), masked by adj, softmax over j
    out  = elu(att @ Wh)

Restructuring used on-device (the key trick):
    exp(LR(z)) = max(e^z, e^{az})                      (a = 0.2 < 1)
    exp(LR(s_i + d_j)) = e^{a s_i} * max(w_i * A_j, B_j)
        with w = e^{(1-a)s}, A = e^{d}, B = e^{a d}
    The row factor e^{a s_i} cancels in the softmax, so the masked
    numerator is   p_ji = adjT_ji * max(w_i A_j, B_j)   — ONE fused
    TensorScalar (mult + max, both per-partition operands) and ONE
    tensor_tensor mask multiply per tile.  numerator and denominator
    come out of a single PE matmul with rhs = [Wh | 1].
    Everything is scaled by 2^-10 (cancels in the softmax ratio) so the
    fp16 N^2 path cannot overflow (max value ~9.2e4/16 << 65504).

Sharding: 8 cores = 2 head-groups x 4 row-groups. Each core owns 4 heads
and 1024 output rows; it computes the full projection for its heads (all
4096 j) and row-parallel attention for its rows.  Per-core inputs are
column-permuted so the core's own rows come first — this keeps the SPMD
program identical across cores (no core-id-dependent addressing).

elu(v) = relu(v) + min(exp(v), 1) - 1.
"""

from contextlib import ExitStack

import numpy as np

import concourse.bass as bass
import concourse.bacc as bacc
import concourse.mybir as mybir
import concourse.tile as tile
from concourse.bass_utils import run_bass_kernel_spmd
from concourse.masks import make_identity

N, F, HID, H = 4096, 512, 64, 8
ALPHA = 0.2
HG, RG = 2, 4                 # head groups x row groups
HPC, RPC = H // HG, N // RG   # 4 heads / 1024 rows per core
NB = N // 128                 # 32 projection row-blocks == j-chunks
FB = F // 128                 # 4 contraction chunks
IB = RPC // 128               # 8 output row-blocks per core
NCORES = 8
SC_BIAS = -float(np.log(1024.0))  # exp(z + SC_BIAS) = exp(z) / 1024

_CACHE = {}


def _build():
    f16, f32 = mybir.dt.float16, mybir.dt.float32
    Alu = mybir.AluOpType
    Act = mybir.ActivationFunctionType

    nc = bacc.Bacc()
    # all inputs are host-swizzled so each SBUF partition reads one long
    # contiguous DRAM run (big DMA descriptors)
    xhi = nc.declare_dram_parameter("xhi", [128, FB, N], f16, isOutput=False)
    xlo = nc.declare_dram_parameter("xlo", [128, FB, N], f16, isOutput=False)
    # [ W(4 heads, 64 each) | wsd_hi(8) | wsd_lo(8) ]
    wext = nc.declare_dram_parameter("wext", [128, FB, 272], f16, isOutput=False)
    adjt = nc.declare_dram_parameter("adjt", [128, NB, RPC], f16, isOutput=False)
    out_t = nc.declare_dram_parameter("out", [HPC, RPC, HID], f32, isOutput=True)

    with ExitStack() as ctx:
        tc = ctx.enter_context(tile.TileContext(nc))
        const = ctx.enter_context(tc.tile_pool(name="const", bufs=1))
        dram = ctx.enter_context(tc.tile_pool(name="dram", bufs=1, space="DRAM"))
        ppool = ctx.enter_context(tc.tile_pool(name="ppool", bufs=4))
        epool = ctx.enter_context(tc.tile_pool(name="epool", bufs=4))
        opool = ctx.enter_context(tc.tile_pool(name="opool", bufs=4))

        wext_sb = const.tile([128, FB, 272], f16)
        xhi_sb = const.tile([128, FB, N], f16)
        xlo_sb = const.tile([128, FB, N], f16)
        adjt_tiles = [
            const.tile([128, 4, RPC], f16, name=f"adjt{g}") for g in range(8)
        ]
        rhs_sb = const.tile([128, HPC, NB, 65], f16)
        sd_sb = const.tile([128, NB, 8], f32)
        A_sb = const.tile([128, NB, HPC], f32)
        B_sb = const.tile([128, NB, HPC], f32)
        wcol_sb = const.tile([128, IB, HPC], f16)
        ident_sb = const.tile([128, 128], f16)
        wct_sb = const.tile([32, 128], f16)
        onehot_sb = const.tile([32, IB * HPC, 128], f16)
        wbc_tiles = [
            const.tile([128, RPC], f16, name=f"wbc{h}") for h in range(HPC)
        ]
        scbias = const.tile([128, 1], f32)
        nc.vector.memset(scbias, SC_BIAS)
        nc.vector.memset(onehot_sb[:, :, :], 0.0)
        # onehot[k, r, m] = 1 iff k == r (selector rows for the w broadcast)
        nc.gpsimd.affine_select(
            out=onehot_sb[:, :, :], in_=onehot_sb[:, :, :],
            compare_op=Alu.not_equal, fill=1.0, base=0,
            pattern=[[-1, IB * HPC], [0, 128]], channel_multiplier=1,
        )
        make_identity(nc, ident_sb[:, :])

        # DMA issue order matters: HWDGE queues drain roughly in issue order.
        # The projection gates everything (A/B/w and the matmul rhs all come
        # from it), so x goes first; the main loop then consumes adjacency
        # chunks in ascending jc order at ~1us/chunk, staying just behind the
        # DMA stream.
        nc.sync.dma_start(out=wext_sb[:, :, :], in_=wext[:, :, :])
        HN = N // 2
        for f in range(FB):
            nc.sync.dma_start(out=xhi_sb[:, f, 0:HN], in_=xhi[:, f, 0:HN])
            nc.sync.dma_start(out=xhi_sb[:, f, HN:N], in_=xhi[:, f, HN:N])
        for f in range(FB):
            nc.sync.dma_start(out=xlo_sb[:, f, 0:HN], in_=xlo[:, f, 0:HN])
            nc.sync.dma_start(out=xlo_sb[:, f, HN:N], in_=xlo[:, f, HN:N])
        for g in range(8):
            nc.sync.dma_start(
                out=adjt_tiles[g][:, :, :], in_=adjt[:, 4 * g:4 * (g + 1), :]
            )
        # softmax-denominator ones column of the matmul rhs
        nc.vector.memset(rhs_sb[:, :, :, 64:65], 1.0)

        # ---- projection + head-0 attention, interleaved ----
        # The PE stream alternates projection blocks with head-0 attention
        # matmuls so neither the DVE nor the PE ever waits for the whole
        # other phase (in-order engine queues).
        psmain = ctx.enter_context(tc.tile_pool(name="psmain", bufs=1, space="PSUM"))
        pswide = tc.alloc_tile_pool(name="pswide", bufs=3, space="PSUM")

        def alloc_head_psums(h):
            # 4 row-blocks packed per PSUM bank; start=True clears the whole
            # bank's has_written bits, so only the bank's first matmul sets it
            # (writes to cleared regions overwrite, then accumulate)
            P0 = psmain.tile([128, 4, 65], f32, tag="P0", name=f"P0_{h}")
            P1 = psmain.tile([128, 4, 65], f32, tag="P1", name=f"P1_{h}")
            return P0, P1

        def emit_pair(h, jc0, P0, P1):
            # two j-chunks per mask op: TS scalars differ per chunk so the
            # fused mult+max stays per-chunk; the mask (elementwise min with
            # adj stored as {0, 60000}) runs once over [128, 2*RPC], halving
            # its fixed overhead.
            g, r = jc0 // 4, jc0 % 4
            t = ppool.tile([128, 2, RPC], f16, tag="t", name=f"t_{h}_{jc0}")
            for q in range(2):
                jc = jc0 + q
                nc.vector.tensor_scalar(
                    t[:, q, :], wbc_tiles[h][:, :],
                    A_sb[:, jc, h:h + 1], B_sb[:, jc, h:h + 1],
                    Alu.mult, Alu.max,
                )
            p = ppool.tile([128, 2, RPC], f16, tag="p", name=f"p_{h}_{jc0}")
            nc.vector.tensor_tensor(
                p[:, :, :], t[:, :, :], adjt_tiles[g][:, r:r + 2, :], Alu.min
            )
            for q in range(2):
                jc = jc0 + q
                for ib in range(IB):
                    P = P0 if ib < 4 else P1
                    nc.tensor.matmul(
                        P[:, ib % 4, :], p[:, q, ib * 128:(ib + 1) * 128],
                        rhs_sb[:, h, jc, :],
                        start=(jc == 0 and ib % 4 == 0),
                        stop=(jc == NB - 1 and ib % 4 == 3),
                        skip_group_check=True,
                    )

        def emit_epilogue(h, P0, P1):
            for ib in range(IB):
                P = (P0 if ib < 4 else P1)[:, ib % 4, :]
                rcp = epool.tile([128, 1], f32, tag="rcp", name=f"rcp{h}_{ib}")
                nc.vector.reciprocal(rcp, P[:, 64:65])
                rt = epool.tile([128, 64], f32, tag="rt", name=f"rt{h}_{ib}")
                nc.scalar.activation(rt, P[:, 0:64], Act.Relu, scale=rcp)
                qt = epool.tile([128, 64], f32, tag="qt", name=f"qt{h}_{ib}")
                nc.scalar.activation(qt, P[:, 0:64], Act.Exp, scale=rcp)
                # elu(v) = relu(v) - relu(1 - e^v)
                mt = epool.tile([128, 64], f32, tag="mt", name=f"mt{h}_{ib}")
                nc.scalar.activation(mt, qt, Act.Relu, scale=-1.0, bias=1.0)
                ot = opool.tile([128, 64], f32, tag="ot", name=f"ot{h}_{ib}")
                nc.vector.tensor_tensor(ot, rt, mt, Alu.subtract)
                nc.sync.dma_start(out=out_t[h, ib * 128:(ib + 1) * 128, :], in_=ot)

        def emit_wbc(h):
            # broadcast head h's w row to all 128 partitions via one-hot matmuls
            for b in range(IB):
                r = b * HPC + h
                bc_ps = pswide.tile([128, 128], f32, tag="bcps", bufs=2)
                nc.tensor.matmul(
                    bc_ps, onehot_sb[:, r, :], wct_sb[:, :], start=True, stop=True
                )
                nc.scalar.activation(
                    wbc_tiles[h][:, b * 128:(b + 1) * 128], bc_ps, Act.Copy
                )

        P0_0, P1_0 = alloc_head_psums(0)
        for nb in range(NB):
            pw = pswide.tile([128, 264], f32)
            for f in range(FB):
                nc.tensor.matmul(
                    pw, xhi_sb[:, f, nb * 128:(nb + 1) * 128], wext_sb[:, f, 0:264],
                    start=(f == 0), stop=False,
                )
            # all three double-fp16 sd terms accumulate into psum cols 256:264,
            # so sd needs no post-add at all
            for f in range(FB):
                nc.tensor.matmul(
                    pw[:, 256:264], xhi_sb[:, f, nb * 128:(nb + 1) * 128],
                    wext_sb[:, f, 264:272], start=False, stop=False,
                )
            for f in range(FB):
                nc.tensor.matmul(
                    pw[:, 256:264], xlo_sb[:, f, nb * 128:(nb + 1) * 128],
                    wext_sb[:, f, 256:264], start=False, stop=(f == FB - 1),
                )
            # sd first (the w/A/B chain is on the critical path), then rhs
            nc.scalar.activation(sd_sb[:, nb, :], pw[:, 256:264], Act.Copy)
            dcols = sd_sb[:, nb:nb + 1, 1:8:2]
            nc.scalar.activation(A_sb[:, nb:nb + 1, :], dcols, Act.Exp)
            nc.scalar.activation(
                B_sb[:, nb:nb + 1, :], dcols, Act.Exp,
                scale=ALPHA, bias=scbias[:, :],
            )
            nc.scalar.activation(
                rhs_sb[:, :, nb, 0:64],
                pw[:, 0:256].rearrange("p (h d) -> p h d", h=HPC),
                Act.Copy,
            )
            if nb == IB - 1:
                # own rows (blocks 0..7 thanks to the permutation): w factors,
                # transpose to one free-dim row — no DMA (a DMA here starves
                # behind the bulk input stream)
                nc.scalar.activation(
                    wcol_sb[:, :, :], sd_sb[:, 0:IB, 0:8:2],
                    Act.Exp, scale=1.0 - ALPHA, bias=scbias[:, :],
                )
                wct_ps = pswide.tile([32, 128], f16, tag="wctps", bufs=1)
                nc.tensor.transpose(
                    wct_ps, wcol_sb[:, :, :].rearrange("p a b -> p (a b)"),
                    ident_sb[:, :],
                )
                nc.scalar.activation(wct_sb[:, :], wct_ps[:, :], Act.Copy)
                emit_wbc(0)
            if nb >= IB - 1 and (nb - (IB - 1)) % 5 == 4:
                # stagger the other heads' broadcasts so they don't delay
                # the projection evacuations
                hh = (nb - (IB - 1)) // 5 + 1
                if hh < HPC:
                    emit_wbc(hh)
            if nb >= IB and nb % 2 == 1:
                # head-0 attention, IB chunks behind the projection: its
                # wbc/rhs/A/B producers must already be emitted (trace order
                # is program order)
                emit_pair(0, nb - IB - 1, P0_0, P1_0)
        pswide.release()
        for jc in range(NB - IB, NB, 2):
            emit_pair(0, jc, P0_0, P1_0)
        emit_epilogue(0, P0_0, P1_0)

        # ---- remaining heads ----
        for h in range(1, HPC):
            P0, P1 = alloc_head_psums(h)
            for jc in range(0, NB, 2):
                emit_pair(h, jc, P0, P1)
            emit_epilogue(h, P0, P1)
    nc.finalize()
    return nc


def _get_nc():
    if "nc" not in _CACHE:
        _CACHE["nc"] = _build()
    return _CACHE["nc"]


def _prepare_in_maps(x, adj, W, a):
    x = np.asarray(x, np.float32)
    adj = np.asarray(adj, np.float32)
    W = np.asarray(W, np.float32)
    a = np.asarray(a, np.float32)
    xT = np.ascontiguousarray(x.T)
    adjT = np.ascontiguousarray(adj.T)
    all_rows = np.arange(N)
    in_maps = []
    for c in range(NCORES):
        hg, rg = divmod(c, RG)
        own = np.arange(rg * RPC, (rg + 1) * RPC)
        perm = np.concatenate([own, np.delete(all_rows, own)])
        xt = xT[:, perm]
        xhi = xt.astype(np.float16)
        xlo = (xt - xhi.astype(np.float32)).astype(np.float16)
        heads = [hg * HPC + h for h in range(HPC)]
        wsd = np.stack(
            sum([[W[gh] @ a[gh, :HID], W[gh] @ a[gh, HID:]] for gh in heads], []),
            axis=1,
        ).astype(np.float32)  # [F, 8] cols (h0 s, h0 d, h1 s, ...)
        wsdh = wsd.astype(np.float16)
        wsdl = (wsd - wsdh.astype(np.float32)).astype(np.float16)
        wext = np.concatenate(
            [W[gh] for gh in heads] + [wsdh, wsdl], axis=1
        ).astype(np.float16)  # [F, 272]
        adjt_c = (adjT[perm][:, own] * 60000.0).astype(np.float16)

        def swz(m):
            # [K*128, M] -> [128, K, M]: partition-major so each SBUF
            # partition reads one contiguous DRAM run
            k = m.shape[0] // 128
            return np.ascontiguousarray(
                m.reshape(k, 128, m.shape[1]).transpose(1, 0, 2)
            )

        in_maps.append({
            "xhi": swz(xhi),
            "xlo": swz(xlo),
            "wext": swz(wext),
            "adjt": swz(adjt_c),
        })
    return in_maps


def _assemble(results):
    full = np.empty((N, H * HID), np.float32)
    for c in range(NCORES):
        hg, rg = divmod(c, RG)
        o = results[c]["out"]  # [HPC, RPC, HID]
        full[rg * RPC:(rg + 1) * RPC, hg * HPC * HID:(hg + 1) * HPC * HID] = (
            o.transpose(1, 0, 2).reshape(RPC, HPC * HID)
        )
    return full


def _run(in_maps, **kw):
    return run_bass_kernel_spmd(_get_nc(), in_maps, list(range(NCORES)), **kw)


def kernel(x, adj, W, a):
    in_maps = _prepare_in_maps(x, adj, W, a)
    res = _run(in_maps)
    return _assemble(res.results)

